# revision 8
# baseline (speedup 1.0000x reference)
"""Trainium2 Bass kernel for nn_MultiHeadAttention_53309134078537.

Reference computation (B=4, S=2048, D=512, H=8, HD=64):
    q = split_heads(Q @ wq + b); k = split_heads(K @ wq + b); v = split_heads(V @ wq + b)
    logits = (q @ k^T) / 8 + pad_mask * (-1e9)
    attn = softmax(logits)          # (B, H, S, S) -- 512 MB fp32, dominates memory traffic
    z = attn @ v; out = merge(z) @ out_kernel + out_bias
    returns (out, attn)

Sharding: 8 cores = (batch b = c//2) x (head-group hg = c%2, 4 heads each).
Per core, attention is computed transposed (logitsT[k, q]) so the padding mask is a
per-partition ACT bias and the z matmul needs no on-chip transpose of the 16.8M-element
attention matrix. Softmax sums come free from a ones-column appended to v; the
normalization 1/sum is exp(-ln(sum)) on ACT, broadcast across partitions by DMA, applied
by DVE in 16-bit 2x mode. attn is written to HBM as fp16 [k, q]; the host transposes to
[q, k] and casts to fp32 (pure data movement). The out-projection partials of the two
head-groups are summed on the host (out_bias is passed as zeros to the hg=1 cores).
"""

import numpy as np

B, S, D, H, HD = 4, 2048, 512, 8, 64
HPC = 4            # heads per core
DHG = HPC * HD     # 256: d_out slice per core
NCORES = 8
SCALE = 1.0 / 8.0
NEG = -1e9 * SCALE  # mask bias applied after the activation scale

P = 128
ST = S // P        # 16 tiles of 128 along sequence
QC = S // 512      # 4 q-chunks of 512
DI = D // P        # 4 tiles of 128 along d_in
DO2 = DHG // P     # 2 tiles of 128 along the core's d_out slice

_CACHE = {}


def _build():
    import concourse.bass as bass
    import concourse.tile as tile
    from concourse import bacc, mybir

    f32, f16 = mybir.dt.float32, mybir.dt.float16
    AF = mybir.ActivationFunctionType

    nc = bacc.Bacc("TRN2", target_bir_lowering=False)

    Q = nc.dram_tensor("q_in", [S, D], f32, kind="ExternalInput")
    K = nc.dram_tensor("k_in", [S, D], f32, kind="ExternalInput")
    V = nc.dram_tensor("v_in", [S, D], f32, kind="ExternalInput")
    MASK = nc.dram_tensor("mask", [1, S], f32, kind="ExternalInput")
    WQ = nc.dram_tensor("wq", [D, DHG], f32, kind="ExternalInput")
    WQB = nc.dram_tensor("wqb", [1, DHG], f32, kind="ExternalInput")
    WO = nc.dram_tensor("wo", [DHG, D], f32, kind="ExternalInput")
    WOB = nc.dram_tensor("wob", [1, D], f32, kind="ExternalInput")
    ATTN = nc.dram_tensor("attn_t", [HPC, S, S], f16, kind="ExternalOutput")
    OUT = nc.dram_tensor("out_p", [S, D], f32, kind="ExternalOutput")

    with tile.TileContext(nc) as tc:
        with (
            tc.tile_pool(name="persist", bufs=1) as persist,
            tc.tile_pool(name="dram", bufs=1, space="DRAM") as dram,
            tc.tile_pool(name="psL", bufs=2, space="PSUM") as psL,
            tc.tile_pool(name="psZ", bufs=2, space="PSUM") as psZ,
            tc.tile_pool(name="psO", bufs=2, space="PSUM") as psO,
            tc.tile_pool(name="work", bufs=2) as work,
            tc.tile_pool(name="exp", bufs=22) as exp_pool,
            tc.tile_pool(name="small", bufs=2) as small,
        ):
            # ---- persistent SBUF state ----
            qT = persist.tile([P, DO2, S], f16)      # q_projT: [dout, s]
            kT = persist.tile([P, DO2, S], f16)      # k_projT
            zT = persist.tile([P, DO2, S], f16)      # zT: rows h*64..h*64+64 per head
            mask_bias = persist.tile([P, ST], f32)   # NEG * mask, partition layout
            wqb_part = persist.tile([P, DO2], f32)   # wq bias, partition layout
            wqb_bc = persist.tile([P, DHG], f32)     # wq bias broadcast along partitions
            wob_bc = persist.tile([P, D], f32)       # out bias broadcast along partitions

            with tc.tile_pool(name="load", bufs=1) as load:
                # fp16 copies of Q/K/V in DRAM so the xbar transpose (16-bit only)
                # can produce QT/KT/VT directly.
                qkv16 = dram.tile([3, S, D], f16)
                for i, s in enumerate((Q, K, V)):
                    nc.gpsimd.dma_start(out=qkv16[i], in_=s.ap())

                wq_sb = load.tile([P, DI, DHG], f16)
                nc.gpsimd.dma_start(
                    out=wq_sb, in_=WQ.ap().rearrange("(t p) n -> p t n", p=P)
                )
                wo_sb = persist.tile([P, DO2, D], f16)
                nc.gpsimd.dma_start(
                    out=wo_sb, in_=WO.ap().rearrange("(t p) n -> p t n", p=P)
                )
                nc.sync.dma_start(
                    out=wqb_part, in_=WQB.ap().rearrange("1 (t p) -> p t", p=P)
                )
                nc.gpsimd.dma_start(out=wqb_bc, in_=WQB.ap().to_broadcast((P, DHG)))
                nc.gpsimd.dma_start(out=wob_bc, in_=WOB.ap().to_broadcast((P, D)))

                mask_part = load.tile([P, ST], f32)
                nc.sync.dma_start(
                    out=mask_part, in_=MASK.ap().rearrange("1 (t p) -> p t", p=P)
                )
                nc.vector.tensor_scalar_mul(out=mask_bias, in0=mask_part, scalar1=NEG)

                vext = persist.tile([P, ST, HPC, HD + 1], f16)
                nc.vector.memset(vext, 0.0)

                # Per tensor: xbar-transpose fp16 DRAM copy -> X^T in SBUF, then
                # project. q/k produce [dout, s]; v produces natural [s, dout]
                # folded into per-head v_ext tiles with a ones column.
                for i, dst in ((0, qT), (1, kT), (2, None)):
                    xTsb = load.tile([P, DI, S], f16, tag="xT", bufs=2, name=f"xT{i}")
                    for t in range(DI):
                        nc.sync.dma_start_transpose(
                            out=xTsb[:, t, :], in_=qkv16[i][:, t * P : (t + 1) * P]
                        )
                    if dst is not None:
                        for dot in range(DO2):
                            for qc in range(QC):
                                ps = psO.tile([P, 512], mybir.dt.float32, tag="o")
                                for di in range(DI):
                                    nc.tensor.matmul(
                                        ps,
                                        lhsT=wq_sb[:, di, dot * P : (dot + 1) * P],
                                        rhs=xTsb[:, di, qc * 512 : (qc + 1) * 512],
                                        start=(di == 0),
                                        stop=(di == DI - 1),
                                    )
                                nc.scalar.activation(
                                    out=dst[:, dot, qc * 512 : (qc + 1) * 512],
                                    in_=ps,
                                    func=AF.Identity,
                                    bias=wqb_part[:, dot : dot + 1],
                                    scale=1.0,
                                )
                    else:
                        for st in range(ST):
                            ps = psO.tile([P, 512], mybir.dt.float32, tag="o")
                            for di in range(DI):
                                nc.tensor.matmul(
                                    ps[:, :DHG],
                                    lhsT=xTsb[:, di, st * P : (st + 1) * P],
                                    rhs=wq_sb[:, di, :],
                                    start=(di == 0),
                                    stop=(di == DI - 1),
                                )
                            for h in range(HPC):
                                nc.vector.tensor_add(
                                    out=vext[:, st, h, :HD],
                                    in0=ps[:, h * HD : (h + 1) * HD],
                                    in1=wqb_bc[:, h * HD : (h + 1) * HD],
                                )
                        nc.vector.memset(vext[:, :, :, HD : HD + 1], 1.0)

            # ---- attention, one head at a time ----
            for h in range(HPC):
                dot, r0 = h // 2, (h % 2) * HD
                expTs = []
                for kt in range(ST):
                    et = exp_pool.tile([P, S], f16, tag="expT", name=f"expT_{h}_{kt}")
                    expTs.append(et)
                    for half in range(2):
                        ps = psL.tile([P, 1024], mybir.dt.float32, tag="l")
                        for j in range(2):
                            c0 = half * 1024 + j * 512
                            nc.tensor.matmul(
                                ps[:, j * 512 : (j + 1) * 512],
                                lhsT=kT[r0 : r0 + HD, dot, kt * P : (kt + 1) * P],
                                rhs=qT[r0 : r0 + HD, dot, c0 : c0 + 512],
                                start=True,
                                stop=True,
                            )
                        nc.scalar.activation(
                            out=et[:, half * 1024 : (half + 1) * 1024],
                            in_=ps,
                            func=AF.Exp,
                            bias=mask_bias[:, kt : kt + 1],
                            scale=SCALE,
                        )

                # z^T (+ sums row) accumulated over k tiles
                zext = work.tile([P, S], f16, tag="zext")
                for qc in range(QC):
                    ps = psZ.tile([P, 512], mybir.dt.float32, tag="z")
                    for kt in range(ST):
                        nc.tensor.matmul(
                            ps[: HD + 1, :],
                            lhsT=vext[:, kt, h, :],
                            rhs=expTs[kt][:, qc * 512 : (qc + 1) * 512],
                            start=(kt == 0),
                            stop=(kt == ST - 1),
                        )
                    nc.vector.tensor_copy(
                        out=zext[: HD + 1, qc * 512 : (qc + 1) * 512],
                        in_=ps[: HD + 1, :],
                    )

                # 1/sum via exp(-ln(sum)), then broadcast to all partitions
                rsum = small.tile([1, S], mybir.dt.float32, tag="rsum")
                nc.scalar.activation(out=rsum, in_=zext[HD : HD + 1, :], func=AF.Ln)
                nc.scalar.activation(out=rsum, in_=rsum, func=AF.Exp, scale=-1.0)
                rsum_dram = dram.tile([1, S], mybir.dt.float32, tag="rsum_d", bufs=2)
                nc.sync.dma_start(out=rsum_dram, in_=rsum)
                rbc = work.tile([P, S], f16, tag="rbc")
                nc.gpsimd.dma_start(out=rbc, in_=rsum_dram.to_broadcast((P, S)))

                # normalize attn^T in place and write out; normalize z^T
                for kt in range(ST):
                    nc.vector.tensor_mul(out=expTs[kt], in0=expTs[kt], in1=rbc)
                    nc.sync.dma_start(
                        out=ATTN.ap()[h, kt * P : (kt + 1) * P, :], in_=expTs[kt]
                    )
                nc.vector.tensor_mul(
                    out=zT[r0 : r0 + HD, dot, :], in0=zext[:HD, :], in1=rbc[:HD, :]
                )

            # ---- out projection ----
            for qt in range(ST):
                ps = psO.tile([P, 512], mybir.dt.float32, tag="o")
                for dt in range(DO2):
                    nc.tensor.matmul(
                        ps,
                        lhsT=zT[:, dt, qt * P : (qt + 1) * P],
                        rhs=wo_sb[:, dt, :],
                        start=(dt == 0),
                        stop=(dt == DO2 - 1),
                    )
                osb = work.tile([P, D], mybir.dt.float32, tag="osb")
                nc.vector.tensor_add(out=osb, in0=ps, in1=wob_bc)
                nc.sync.dma_start(out=OUT.ap()[qt * P : (qt + 1) * P, :], in_=osb)

    nc.finalize()
    return nc


def kernel(Q, K, V, pad_mask, wq_kernel, wq_bias, out_kernel, out_bias, **run_kwargs):
    from concourse.bass_utils import run_bass_kernel_spmd

    if "nc" not in _CACHE:
        _CACHE["nc"] = _build()
    nc = _CACHE["nc"]

    in_maps = []
    for c in range(NCORES):
        b, hg = c // 2, c % 2
        hs = slice(hg * DHG, (hg + 1) * DHG)
        in_maps.append(
            {
                "q_in": np.ascontiguousarray(Q[b], dtype=np.float32),
                "k_in": np.ascontiguousarray(K[b], dtype=np.float32),
                "v_in": np.ascontiguousarray(V[b], dtype=np.float32),
                "mask": np.ascontiguousarray(
                    pad_mask[b, 0, 0, :][None, :], dtype=np.float32
                ),
                "wq": np.ascontiguousarray(wq_kernel[:, hs], dtype=np.float32),
                "wqb": np.ascontiguousarray(wq_bias[hs][None, :], dtype=np.float32),
                "wo": np.ascontiguousarray(out_kernel[hs, :], dtype=np.float32),
                "wob": np.ascontiguousarray(
                    (out_bias if hg == 0 else np.zeros_like(out_bias))[None, :],
                    dtype=np.float32,
                ),
            }
        )

    res = run_bass_kernel_spmd(nc, in_maps, core_ids=list(range(NCORES)), **run_kwargs)
    results = res.results if hasattr(res, "results") else res

    out = np.empty((B, S, D), dtype=np.float32)
    attn = np.empty((B, H, S, S), dtype=np.float32)
    for c in range(NCORES):
        b, hg = c // 2, c % 2
        at = results[c]["attn_t"]  # fp16 [HPC, S(k), S(q)]
        for i in range(HPC):
            attn[b, hg * HPC + i] = at[i].T
    for b in range(B):
        out[b] = results[2 * b]["out_p"] + results[2 * b + 1]["out_p"]
    if "trace" in run_kwargs:
        _CACHE["last_run"] = res
    return out, attn


# revision 10
# speedup vs baseline: 1.0138x; 1.0138x over previous
"""Trainium2 Bass kernel for nn_MultiHeadAttention_53309134078537.

Reference computation (B=4, S=2048, D=512, H=8, HD=64):
    q = split_heads(Q @ wq + b); k = split_heads(K @ wq + b); v = split_heads(V @ wq + b)
    logits = (q @ k^T) / 8 + pad_mask * (-1e9)
    attn = softmax(logits)          # (B, H, S, S) -- 512 MB fp32, dominates memory traffic
    z = attn @ v; out = merge(z) @ out_kernel + out_bias
    returns (out, attn)

Sharding: 8 cores = (batch b = c//2) x (head-group hg = c%2, 4 heads each).
Per core, attention is computed transposed (logitsT[k, q]) so the padding mask is a
per-partition ACT bias and the z matmul needs no on-chip transpose of the 16.8M-element
attention matrix. Softmax sums come free from a ones-column appended to v; the
normalization 1/sum is exp(-ln(sum)) on ACT, broadcast across partitions by DMA, applied
by DVE in 16-bit 2x mode. Heads are software-pipelined (head h's logits/exp overlap head
h-1's z/normalize/writeback) to keep PE dense and HAM warm. attn is written to HBM as
fp16 [k, q]; the host transposes to [q, k] and casts to fp32 (pure data movement). The
out-projection partials of the two head-groups are summed on the host (out_bias is
passed as zeros to the hg=1 cores).
"""

import numpy as np

B, S, D, H, HD = 4, 2048, 512, 8, 64
HPC = 4            # heads per core
DHG = HPC * HD     # 256: d_out slice per core
NCORES = 8
SCALE = 1.0 / 8.0
NEG = -1e9 * SCALE  # mask bias applied after the activation scale

P = 128
ST = S // P        # 16 tiles of 128 along sequence
QC = S // 512      # 4 q-chunks of 512
DI = D // P        # 4 tiles of 128 along d_in
DO2 = DHG // P     # 2 tiles of 128 along the core's d_out slice

_CACHE = {}


def _build():
    import concourse.bass as bass
    import concourse.tile as tile
    from concourse import bacc, mybir

    f32, f16 = mybir.dt.float32, mybir.dt.float16
    AF = mybir.ActivationFunctionType
    ALU = mybir.AluOpType

    nc = bacc.Bacc("TRN2", target_bir_lowering=False)

    Q = nc.dram_tensor("q_in", [S, D], f32, kind="ExternalInput")
    K = nc.dram_tensor("k_in", [S, D], f32, kind="ExternalInput")
    V = nc.dram_tensor("v_in", [S, D], f32, kind="ExternalInput")
    MASK = nc.dram_tensor("mask", [1, S], f32, kind="ExternalInput")
    WQ = nc.dram_tensor("wq", [D, DHG], f32, kind="ExternalInput")
    WQB = nc.dram_tensor("wqb", [1, DHG], f32, kind="ExternalInput")
    WO = nc.dram_tensor("wo", [DHG, D], f32, kind="ExternalInput")
    WOB = nc.dram_tensor("wob", [1, D], f32, kind="ExternalInput")
    ATTN = nc.dram_tensor("attn_t", [HPC, S, S], f16, kind="ExternalOutput")
    OUT = nc.dram_tensor("out_p", [S, D], f32, kind="ExternalOutput")

    with tile.TileContext(nc) as tc:
        with (
            tc.tile_pool(name="persist", bufs=1) as persist,
            tc.tile_pool(name="dram", bufs=1, space="DRAM") as dram,
            tc.tile_pool(name="psL", bufs=3, space="PSUM") as psL,
            tc.tile_pool(name="psZ", bufs=2, space="PSUM") as psZ,
            tc.tile_pool(name="work", bufs=2) as work,
        ):
            # ---- persistent SBUF state ----
            qT = persist.tile([P, DO2, S], f16)      # q_projT: [dout, s]
            kT = persist.tile([P, DO2, S], f16)      # k_projT
            zT = persist.tile([P, DO2, S], f16)      # zT: rows h*64..h*64+64 per head
            vext = persist.tile([P, ST, HPC, HD + 1], f16)
            wo_sb = persist.tile([P, DO2, D], f16)
            mask_bias = persist.tile([P, ST], f32)   # NEG * mask, partition layout
            wqb_part = persist.tile([P, DO2], f32)   # wq bias, partition layout
            wqb_bc = persist.tile([P, DHG], f16)     # wq bias broadcast along partitions
            wob_bc = persist.tile([P, D], f16)       # out bias broadcast along partitions

            with tc.tile_pool(name="load", bufs=1) as load:
                wq_sb = load.tile([P, DI, DHG], f16)
                nc.gpsimd.dma_start(
                    out=wq_sb, in_=WQ.ap().rearrange("(t p) n -> p t n", p=P)
                )
                nc.gpsimd.dma_start(
                    out=wo_sb, in_=WO.ap().rearrange("(t p) n -> p t n", p=P)
                )
                nc.sync.dma_start(
                    out=wqb_part, in_=WQB.ap().rearrange("1 (t p) -> p t", p=P)
                )
                nc.gpsimd.dma_start(out=wqb_bc, in_=WQB.ap().to_broadcast((P, DHG)))
                nc.gpsimd.dma_start(out=wob_bc, in_=WOB.ap().to_broadcast((P, D)))

                mask_part = load.tile([P, ST], f32)
                nc.sync.dma_start(
                    out=mask_part, in_=MASK.ap().rearrange("1 (t p) -> p t", p=P)
                )
                nc.vector.tensor_scalar_mul(out=mask_bias, in0=mask_part, scalar1=NEG)

                nc.vector.memset(vext, 0.0)

                # Per tensor: SWDGE cast to fp16 DRAM scratch, xbar-transpose to
                # X^T in SBUF, project. q/k produce [dout, s]; v lands in the
                # per-head v_ext tiles (ones column added after).
                for i, (src, dst) in enumerate(((Q, qT), (K, kT), (V, None))):
                    x16 = dram.tile([S, D], f16, tag="x16", bufs=3, name=f"x16_{i}")
                    nc.gpsimd.dma_start(out=x16, in_=src.ap())
                    xTsb = load.tile([P, DI, S], f16, tag="xT", bufs=2, name=f"xT{i}")
                    for t in range(DI):
                        nc.sync.dma_start_transpose(
                            out=xTsb[:, t, :], in_=x16[:, t * P : (t + 1) * P]
                        )
                    if dst is not None:
                        for dot in range(DO2):
                            for qc in range(QC):
                                ps = psL.tile([P, 1024], f32, tag="l")
                                for di in range(DI):
                                    nc.tensor.matmul(
                                        ps[:, :512],
                                        lhsT=wq_sb[:, di, dot * P : (dot + 1) * P],
                                        rhs=xTsb[:, di, qc * 512 : (qc + 1) * 512],
                                        start=(di == 0),
                                        stop=(di == DI - 1),
                                    )
                                nc.vector.tensor_scalar(
                                    out=dst[:, dot, qc * 512 : (qc + 1) * 512],
                                    in0=ps[:, :512],
                                    scalar1=wqb_part[:, dot : dot + 1],
                                    scalar2=None,
                                    op0=ALU.add,
                                )
                    else:
                        for st in range(ST):
                            ps = psL.tile([P, 1024], f32, tag="l")
                            for di in range(DI):
                                nc.tensor.matmul(
                                    ps[:, :DHG],
                                    lhsT=xTsb[:, di, st * P : (st + 1) * P],
                                    rhs=wq_sb[:, di, :],
                                    start=(di == 0),
                                    stop=(di == DI - 1),
                                )
                            for h in range(HPC):
                                nc.vector.tensor_add(
                                    out=vext[:, st, h, :HD],
                                    in0=ps[:, h * HD : (h + 1) * HD],
                                    in1=wqb_bc[:, h * HD : (h + 1) * HD],
                                )
                        nc.vector.memset(vext[:, :, :, HD : HD + 1], 1.0)

            # ---- attention: heads software-pipelined ----
            with tc.tile_pool(name="exp", bufs=34) as exp_pool:
                expT_all = {}

                def emit_logits_exp(h):
                    dot, r0 = h // 2, (h % 2) * HD
                    tiles = []
                    for kt in range(ST):
                        et = exp_pool.tile(
                            [P, S], f16, tag="expT", name=f"expT_{h}_{kt}"
                        )
                        tiles.append(et)
                        for half in range(2):
                            ps = psL.tile([P, 1024], f32, tag="l")
                            for j in range(2):
                                c0 = half * 1024 + j * 512
                                nc.tensor.matmul(
                                    ps[:, j * 512 : (j + 1) * 512],
                                    lhsT=kT[r0 : r0 + HD, dot, kt * P : (kt + 1) * P],
                                    rhs=qT[r0 : r0 + HD, dot, c0 : c0 + 512],
                                    start=True,
                                    stop=True,
                                )
                            nc.scalar.activation(
                                out=et[:, half * 1024 : (half + 1) * 1024],
                                in_=ps,
                                func=AF.Exp,
                                bias=mask_bias[:, kt : kt + 1],
                                scale=SCALE,
                            )
                    expT_all[h] = tiles

                def emit_tail(h):
                    dot, r0 = h // 2, (h % 2) * HD
                    tiles = expT_all.pop(h)
                    # z^T (+ sums row) accumulated over k tiles; zext rows:
                    # 0..63 z, 64 sum, 65 ln(sum), 66 1/sum
                    zext = work.tile([P, S], mybir.dt.float32, tag="zext")
                    for qc in range(QC):
                        ps = psZ.tile([P, 512], mybir.dt.float32, tag="z")
                        for kt in range(ST):
                            nc.tensor.matmul(
                                ps[: HD + 1, :],
                                lhsT=vext[:, kt, h, :],
                                rhs=tiles[kt][:, qc * 512 : (qc + 1) * 512],
                                start=(kt == 0),
                                stop=(kt == ST - 1),
                            )
                        nc.vector.tensor_copy(
                            out=zext[: HD + 1, qc * 512 : (qc + 1) * 512],
                            in_=ps[: HD + 1, :],
                        )
                    # ACT partition bases must be in {0,32,64,96}: ln goes to
                    # row 96, 1/sum overwrites the sum row at 64.
                    nc.scalar.activation(
                        out=zext[96:97, :], in_=zext[HD : HD + 1, :], func=AF.Ln
                    )
                    nc.scalar.activation(
                        out=zext[HD : HD + 1, :],
                        in_=zext[96:97, :],
                        func=AF.Exp,
                        scale=-1.0,
                    )
                    rsum_dram = dram.tile(
                        [1, S], mybir.dt.float32, tag="rsum_d", bufs=2
                    )
                    nc.sync.dma_start(out=rsum_dram, in_=zext[HD : HD + 1, :])
                    rbc = work.tile([P, S], f16, tag="rbc")
                    nc.gpsimd.dma_start(out=rbc, in_=rsum_dram.to_broadcast((P, S)))

                    for kt in range(ST):
                        nc.vector.tensor_mul(out=tiles[kt], in0=tiles[kt], in1=rbc)
                        nc.sync.dma_start(
                            out=ATTN.ap()[h, kt * P : (kt + 1) * P, :], in_=tiles[kt]
                        )
                    nc.vector.tensor_mul(
                        out=zT[r0 : r0 + HD, dot, :], in0=zext[:HD, :], in1=rbc[:HD, :]
                    )

                for h in range(HPC):
                    emit_logits_exp(h)
                    if h > 0:
                        emit_tail(h - 1)
                emit_tail(HPC - 1)

                # ---- out projection ----
                for qt in range(ST):
                    ps = psL.tile([P, 1024], mybir.dt.float32, tag="l")
                    for dt in range(DO2):
                        nc.tensor.matmul(
                            ps[:, :512],
                            lhsT=zT[:, dt, qt * P : (qt + 1) * P],
                            rhs=wo_sb[:, dt, :],
                            start=(dt == 0),
                            stop=(dt == DO2 - 1),
                        )
                    osb = work.tile([P, D], mybir.dt.float32, tag="osb")
                    nc.vector.tensor_add(out=osb, in0=ps[:, :512], in1=wob_bc)
                    nc.sync.dma_start(out=OUT.ap()[qt * P : (qt + 1) * P, :], in_=osb)

    nc.finalize()
    return nc


def kernel(Q, K, V, pad_mask, wq_kernel, wq_bias, out_kernel, out_bias, **run_kwargs):
    from concourse.bass_utils import run_bass_kernel_spmd

    if "nc" not in _CACHE:
        _CACHE["nc"] = _build()
    nc = _CACHE["nc"]

    in_maps = []
    for c in range(NCORES):
        b, hg = c // 2, c % 2
        hs = slice(hg * DHG, (hg + 1) * DHG)
        in_maps.append(
            {
                "q_in": np.ascontiguousarray(Q[b], dtype=np.float32),
                "k_in": np.ascontiguousarray(K[b], dtype=np.float32),
                "v_in": np.ascontiguousarray(V[b], dtype=np.float32),
                "mask": np.ascontiguousarray(
                    pad_mask[b, 0, 0, :][None, :], dtype=np.float32
                ),
                "wq": np.ascontiguousarray(wq_kernel[:, hs], dtype=np.float32),
                "wqb": np.ascontiguousarray(wq_bias[hs][None, :], dtype=np.float32),
                "wo": np.ascontiguousarray(out_kernel[hs, :], dtype=np.float32),
                "wob": np.ascontiguousarray(
                    (out_bias if hg == 0 else np.zeros_like(out_bias))[None, :],
                    dtype=np.float32,
                ),
            }
        )

    res = run_bass_kernel_spmd(nc, in_maps, core_ids=list(range(NCORES)), **run_kwargs)
    results = res.results if hasattr(res, "results") else res

    out = np.empty((B, S, D), dtype=np.float32)
    attn = np.empty((B, H, S, S), dtype=np.float32)
    for c in range(NCORES):
        b, hg = c // 2, c % 2
        at = results[c]["attn_t"]  # fp16 [HPC, S(k), S(q)]
        for i in range(HPC):
            attn[b, hg * HPC + i] = at[i].T
    for b in range(B):
        out[b] = results[2 * b]["out_p"] + results[2 * b + 1]["out_p"]
    if "trace" in run_kwargs:
        _CACHE["last_run"] = res
    return out, attn
